# revision 10
# baseline (speedup 1.0000x reference)
"""Trainium2 Bass kernel for nn_BalancedTreeCell (binary-tree GNN message passing).

Math per batch row (data parallel over N=16 rows, 2 rows per core on 8 cores):

  state = LN(input @ w_word + b_word)                       [S, D]
  repeat log2(S) times:
     l, r    = state[0::2], state[1::2]
     h       = gelu([l r] @ w1 + b1)                        [S/2, H]
     c       = h @ w2 + b2   -> f1,f2,i = sigmoid(c[:3D]), parent = c[3D:]
     state   = LN(f1*l + f2*r + i*parent)                   [S/2, D]
  out = state[0]                                            [D]

Implementation notes (v2, bf16):
 - Whole datapath in bf16 (PSUM accumulation fp32).  bf16 moving operands
   stream at 1 cycle/row on the PE at 2.4 GHz (fp32r is xbus-limited to
   ~2 cyc/row), halving matmul time vs the fp32r version.
 - sigmoid(x) = (1+tanh(x/2))/2 and LayerNorm scale-invariance let the whole
   kernel use ONE activation table set (gelu_and_others: gelu, tanh,
   identity, copy) - zero ACT_TABLE_LOAD switches in steady state.
   z' = (l+r+par) + tanh(c1/2)*l + tanh(c2/2)*r + tanh(c3/2)*par = 2*z and
   LN(2z) = LN(z).
 - Input is pre-transposed to feature-major on the host: no input transposes.
 - State is stored de-interleaved ([evens | odds]) at every level so l/r
   operands are contiguous (full-rate bf16 DVE/PE access).
 - LayerNorm stats for a batch of up to 4 groups go into a [32,128] PSUM
   block (4 col-blocks per group); one fp32 DVE chain incl. a bit-trick +
   2-step-Newton rsqrt entirely on DVE serves the batch.  LN apply uses K=1
   broadcast matmuls with the LN gain folded into the stationary row.
"""

import numpy as np
import ml_dtypes

import concourse.bass as bass
import concourse.bacc as bacc
import concourse.tile as tile
import concourse.mybir as mybir
from concourse.bass_utils import run_bass_kernel_spmd

F32 = mybir.dt.float32
BF16 = mybir.dt.bfloat16
U32 = mybir.dt.uint32
AF = mybir.ActivationFunctionType
ALU = mybir.AluOpType

P = 128
N_TOT = 16
S_FULL = 4096
D = 256
H = 1024
N_CORES = 8
R = N_TOT // N_CORES          # rows per core
DPT = D // P                  # 2 feature partition-tiles
EPS = 1e-5
G = 512                       # token group (PSUM bank = 512 fp32)
BATCH = 3                     # LN batch (stats rows live at partitions 0/32/64)

# rsqrt seed: r0 = bitcast_f32(~(bits(v)>>1)) * RC  (~4.3% rel err)
RC = -1.8369135901441048e-20


def _build(S=S_FULL, iters=1):
    nc = bacc.Bacc("TRN2", target_bir_lowering=False, debug=False)
    T0 = R * S

    # ---- DRAM I/O (per core) ----
    x_d = nc.dram_tensor("x", [DPT, P, T0], BF16, kind="ExternalInput").ap()
    ww_d = nc.dram_tensor("wwt", [P, DPT, D], BF16, kind="ExternalInput").ap()
    bw_d = nc.dram_tensor("bwc", [P, DPT], F32, kind="ExternalInput").ap()
    lnr_d = nc.dram_tensor("lnrow", [4, 65, D], BF16, kind="ExternalInput").ap()
    lnb_d = nc.dram_tensor("lnbcol", [P, 2, 2], F32, kind="ExternalInput").ap()
    ones_d = nc.dram_tensor("onescol", [P, 1], BF16, kind="ExternalInput").ap()
    w1_d = nc.dram_tensor("w1t", [P, 4, H], BF16, kind="ExternalInput").ap()
    b1_d = nc.dram_tensor("b1c", [P, 8], F32, kind="ExternalInput").ap()
    w2_d = nc.dram_tensor("w2t", [P, 8, H], BF16, kind="ExternalInput").ap()
    b2_d = nc.dram_tensor("b2c", [P, 8], F32, kind="ExternalInput").ap()
    b2h_d = nc.dram_tensor("b2h", [P, 8], F32, kind="ExternalInput").ap()
    eye_d = nc.dram_tensor("eye", [P, P], BF16, kind="ExternalInput").ap()
    onesr_d = nc.dram_tensor("onesrow", [1, P], BF16, kind="ExternalInput").ap()
    b1r_d = nc.dram_tensor("b1row", [1, H], BF16, kind="ExternalInput").ap()
    b2r_d = nc.dram_tensor("b2row", [1, H], BF16, kind="ExternalInput").ap()
    gcr_d = nc.dram_tensor("gcrep", [P, D], BF16, kind="ExternalInput").ap()
    bcr_d = nc.dram_tensor("bcrep", [P, D], BF16, kind="ExternalInput").ap()
    out_d = nc.dram_tensor("out", [R, D], F32, kind="ExternalOutput").ap()
    import os
    PROBE = os.environ.get("KPROBE", "")
    probe_d = (nc.dram_tensor("probe", [P, 4, 512], BF16, kind="ExternalOutput").ap()
               if PROBE else None)

    xr = x_d.rearrange("a p t -> p a t")   # [P, DPT, T0]

    with tile.TileContext(nc) as tc:
        cst = tc.alloc_tile_pool(name="cst", bufs=1)
        stp = tc.alloc_tile_pool(name="stp", bufs=1)
        xin = tc.alloc_tile_pool(name="xin", bufs=2)
        hsb = tc.alloc_tile_pool(name="hsb", bufs=2)
        gsb = tc.alloc_tile_pool(name="gsb", bufs=2)
        zp = tc.alloc_tile_pool(name="zp", bufs=6)
        zq = tc.alloc_tile_pool(name="zq", bufs=2)
        ch = tc.alloc_tile_pool(name="ch", bufs=1)
        rows = tc.alloc_tile_pool(name="rows", bufs=2)
        tl = tc.alloc_tile_pool(name="tl", bufs=1)
        ps_ph = tc.alloc_tile_pool(name="psph", bufs=2, space="PSUM")
        ps_pc = tc.alloc_tile_pool(name="pspc", bufs=2, space="PSUM")
        ps_st = tc.alloc_tile_pool(name="psst", bufs=1, space="PSUM")
        ps_bc = tc.alloc_tile_pool(name="psbc", bufs=2, space="PSUM")

        # ---- constants (early-needed first; w1/w2 overlap with stage 0) ----
        wws = cst.tile([P, DPT, D], BF16)
        nc.sync.dma_start(out=wws, in_=ww_d)
        bws = cst.tile([P, DPT], F32)
        nc.sync.dma_start(out=bws, in_=bw_d)
        lnrs = []
        for i in range(4):
            lt = cst.tile([65, D], BF16, name=f"lnr{i}")
            nc.sync.dma_start(out=lt, in_=lnr_d[i])
            lnrs.append(lt)
        lnbs = cst.tile([P, 2, 2], F32)
        nc.sync.dma_start(out=lnbs, in_=lnb_d)
        oness = cst.tile([P, 1], BF16)
        nc.sync.dma_start(out=oness, in_=ones_d)
        w1s = cst.tile([P, 4, H], BF16)
        nc.sync.dma_start(out=w1s, in_=w1_d)
        b1s = cst.tile([P, 8], F32)
        nc.sync.dma_start(out=b1s, in_=b1_d)
        w2s = cst.tile([P, 8, H], BF16)
        nc.sync.dma_start(out=w2s, in_=w2_d)
        b2s = cst.tile([P, 8], F32)
        nc.sync.dma_start(out=b2s, in_=b2_d)
        b2hs = cst.tile([P, 8], F32)
        nc.sync.dma_start(out=b2hs, in_=b2h_d)
        eyes = cst.tile([P, P], BF16)
        nc.sync.dma_start(out=eyes, in_=eye_d)
        onesr = cst.tile([1, P], BF16)
        nc.sync.dma_start(out=onesr, in_=onesr_d)
        b1row = cst.tile([1, H], BF16)
        nc.sync.dma_start(out=b1row, in_=b1r_d)
        b2row = cst.tile([1, H], BF16)
        nc.sync.dma_start(out=b2row, in_=b2r_d)
        gcrep = cst.tile([P, D], BF16)
        nc.sync.dma_start(out=gcrep, in_=gcr_d)
        bcrep = cst.tile([P, D], BF16)
        nc.sync.dma_start(out=bcrep, in_=bcr_d)

        # ---- persistent state buffers (feature-major, [E | O] layout) ----
        s0out = stp.tile([P, DPT, T0], BF16, name="s0out")
        sA = stp.tile([P, DPT, T0 // 2], BF16, name="sA")
        sB = stp.tile([P, DPT, T0 // 4], BF16, name="sB")

        def body():
            # ---------- batched-LN machinery ----------
            pend = {"items": None}

            def open_batch(Tg):
                pend["st"] = ps_st.tile([P, G], F32, tag="st", name="st")
                pend["sq"] = ps_st.tile([P, G], F32, tag="sq", name="sq")
                pend["Tg"] = Tg
                pend["items"] = []

            def emit_stats(z, zs, Tg, brow):
                o = pend["st"][32 * brow:32 * brow + 1, :Tg]
                for pt in range(DPT):
                    nc.tensor.matmul(o, lhsT=oness, rhs=z[:, pt, :Tg],
                                     start=(pt == 0), stop=(pt == DPT - 1))
                o = pend["sq"][32 * brow:32 * brow + 1, :Tg]
                for pt in range(DPT):
                    nc.tensor.matmul(o, lhsT=oness, rhs=zs[:, pt, :Tg],
                                     start=(pt == 0), stop=(pt == DPT - 1))

            def add_item(z, zs, ln, Tg, writer):
                if pend["items"] is None:
                    open_batch(Tg)
                brow = len(pend["items"])
                emit_stats(z, zs, Tg, brow)
                pend["items"].append(dict(ln=ln, writer=writer))

            def batch_full():
                return (pend["items"] is not None
                        and len(pend["items"]) >= BATCH)

            def flush_batch():
                items = pend["items"]
                if not items:
                    pend["items"] = None
                    return
                Tg = pend["Tg"]
                np_ = 32 * (len(items) - 1) + 1
                stt = pend["st"]
                sqt = pend["sq"]
                mu = ch.tile([P, G], F32, tag="mu", name="mu")[:np_, :Tg]
                nc.vector.tensor_scalar(out=mu, in0=stt[:np_, :Tg],
                                        scalar1=1.0 / D,
                                        scalar2=None, op0=ALU.mult)
                mq = ch.tile([P, G], F32, tag="mq", name="mq")[:np_, :Tg]
                nc.vector.tensor_mul(mq, mu, mu)
                v3 = ch.tile([P, G], F32, tag="v3", name="v3")[:np_, :Tg]
                nc.vector.scalar_tensor_tensor(
                    out=v3, in0=sqt[:np_, :Tg], scalar=1.0 / D, in1=mq,
                    op0=ALU.mult, op1=ALU.subtract)
                nc.vector.tensor_scalar(out=v3, in0=v3, scalar1=EPS,
                                        scalar2=None, op0=ALU.add)
                sd = ch.tile([P, G], F32, tag="sd", name="sd")[:np_, :Tg]
                nc.vector.tensor_scalar(
                    out=sd.bitcast(U32), in0=v3.bitcast(U32),
                    scalar1=1, scalar2=0xFFFFFFFF,
                    op0=ALU.logical_shift_right, op1=ALU.bitwise_xor)
                nc.vector.tensor_scalar(out=sd, in0=sd, scalar1=RC,
                                        scalar2=None, op0=ALU.mult)
                u_ = ch.tile([P, G], F32, tag="u_", name="u_")[:np_, :Tg]
                t_ = ch.tile([P, G], F32, tag="t_", name="t_")[:np_, :Tg]
                q_ = ch.tile([P, G], F32, tag="q_", name="q_")[:np_, :Tg]
                nc.vector.tensor_mul(u_, sd, sd)
                nc.vector.tensor_mul(t_, v3, u_)
                nc.vector.tensor_scalar(out=q_, in0=t_, scalar1=-0.5,
                                        scalar2=1.5, op0=ALU.mult, op1=ALU.add)
                r1 = ch.tile([P, G], F32, tag="r1", name="r1")[:np_, :Tg]
                nc.vector.tensor_mul(r1, q_, sd)
                nc.vector.tensor_mul(u_, r1, r1)
                nc.vector.tensor_mul(t_, v3, u_)
                nc.vector.tensor_scalar(out=q_, in0=t_, scalar1=-0.5,
                                        scalar2=1.5, op0=ALU.mult, op1=ALU.add)
                rsig = rows.tile([P, G], BF16, tag="rsig",
                                 name="rsig")[:np_, :Tg]
                nc.vector.tensor_mul(rsig, q_, r1)
                ms = rows.tile([P, G], BF16, tag="ms", name="ms")[:np_, :Tg]
                nc.vector.tensor_mul(ms, mu, rsig)
                for j, it in enumerate(items):
                    ln = it["ln"]
                    for pt in range(DPT):
                        b1p = ps_bc.tile([P, G], F32, tag="bc",
                                         name="b1p")[:, :Tg]
                        b2p = ps_bc.tile([P, G], F32, tag="bc",
                                         name="b2p")[:, :Tg]
                        nc.tensor.matmul(
                            b1p,
                            lhsT=lnrs[2 * ln][32 * j:32 * j + 1,
                                              pt * P:(pt + 1) * P],
                            rhs=rsig[32 * j:32 * j + 1, :Tg],
                            start=True, stop=True)
                        nc.tensor.matmul(
                            b2p,
                            lhsT=lnrs[2 * ln + 1][32 * j:32 * j + 1,
                                                  pt * P:(pt + 1) * P],
                            rhs=ms[32 * j:32 * j + 1, :Tg],
                            start=True, stop=True)
                        it["writer"](pt, b1p, b2p)
                pend["items"] = None

            def mk_writer(z, dst, To, g, Tg, ln):
                """LN apply + de-interleaved ([E|O]) store of one group."""
                half = Tg // 2
                base = g * half
                obase = To // 2 + base

                def w(pt, b1p, b2p):
                    t = gsb.tile([P, G], F32, tag="tap", name="tap")[:, :Tg]
                    nc.vector.tensor_mul(t, z[:, pt, :Tg], b1p)
                    nc.vector.scalar_tensor_tensor(
                        out=dst[:, pt, base:base + half],
                        in0=t[:, 0:Tg:2], scalar=lnbs[:, ln, pt:pt + 1],
                        in1=b2p[:, 0:Tg:2], op0=ALU.add, op1=ALU.add)
                    nc.vector.scalar_tensor_tensor(
                        out=dst[:, pt, obase:obase + half],
                        in0=t[:, 1:Tg:2], scalar=lnbs[:, ln, pt:pt + 1],
                        in1=b2p[:, 1:Tg:2], op0=ALU.add, op1=ALU.add)
                return w

            # ---------- stage 0 unit: load + word-linear ----------
            def s0_unit(s):
                x0 = xin.tile([P, DPT, G], BF16, tag="x0", name="x0")
                nc.sync.dma_start(out=x0, in_=xr[:, :, s * G:(s + 1) * G])
                z0 = zp.tile([P, DPT, G], BF16, tag="z", name="z0")
                for pt in range(DPT):
                    pw = ps_ph.tile([P, G], F32, tag="ph", name="pw")
                    for k in range(DPT):
                        nc.tensor.matmul(pw,
                                         lhsT=wws[:, k, pt * P:(pt + 1) * P],
                                         rhs=x0[:, k, :],
                                         start=(k == 0), stop=(k == DPT - 1))
                    nc.scalar.activation(out=z0[:, pt, :], in_=pw,
                                         func=AF.Identity,
                                         bias=bws[:, pt:pt + 1])
                zs = zq.tile([P, DPT, G], BF16, tag="zsq", name="zsq")
                nc.vector.tensor_mul(zs, z0, z0)
                return z0, zs

            # ---------- tree cell (feature-major group) ----------
            def cell_M(prev, Tin, g, Tg):
                hf = Tin // 2
                l_ = [prev[:, pt, g * Tg:(g + 1) * Tg] for pt in range(DPT)]
                r_ = [prev[:, pt, hf + g * Tg:hf + (g + 1) * Tg]
                      for pt in range(DPT)]
                xk = [l_[0], l_[1], r_[0], r_[1]]
                h = hsb.tile([P, 8, G], BF16, tag="h", name="h")[:, :, :Tg]
                for m in range(8):
                    ph = ps_ph.tile([P, G], F32, tag="ph", name="ph")[:, :Tg]
                    for k in range(4):
                        nc.tensor.matmul(ph,
                                         lhsT=w1s[:, k, m * P:(m + 1) * P],
                                         rhs=xk[k], start=(k == 0),
                                         stop=(k == 3))
                    nc.scalar.activation(out=h[:, m, :], in_=ph, func=AF.Gelu,
                                         bias=b1s[:, m:m + 1])
                z = zp.tile([P, DPT, G], BF16, tag="z", name="z")[:, :, :Tg]
                par = [None, None]
                s2 = [None, None]
                for m2 in (6, 7, 0, 1, 2, 3, 4, 5):
                    pc = ps_pc.tile([P, G], F32, tag="pc", name="pc")[:, :Tg]
                    for k in range(8):
                        nc.tensor.matmul(pc,
                                         lhsT=w2s[:, k, m2 * P:(m2 + 1) * P],
                                         rhs=h[:, k, :], start=(k == 0),
                                         stop=(k == 7))
                    part, pt = divmod(m2, 2)
                    if part == 3:   # parent ( + l + r sum tree on DVE)
                        pr = gsb.tile([P, G], BF16, tag=f"par{pt}",
                                      name=f"par{pt}")[:, :Tg]
                        nc.scalar.activation(out=pr, in_=pc, func=AF.Identity,
                                             bias=b2s[:, m2:m2 + 1])
                        par[pt] = pr
                        s1 = gsb.tile([P, G], BF16, tag=f"s1{pt}",
                                      name=f"s1{pt}")[:, :Tg]
                        nc.vector.tensor_add(s1, l_[pt], r_[pt])
                        s2t = gsb.tile([P, G], BF16, tag=f"s2{pt}",
                                       name=f"s2{pt}")[:, :Tg]
                        nc.vector.tensor_add(s2t, s1, pr)
                        s2[pt] = s2t
                    else:
                        gt = gsb.tile([P, G], BF16, tag=f"gt{part}{pt}",
                                      name=f"gt{part}{pt}")[:, :Tg]
                        nc.scalar.activation(out=gt, in_=pc, func=AF.Tanh,
                                             bias=b2hs[:, m2:m2 + 1],
                                             scale=0.5)
                        if part == 0:     # + tanh1 * l
                            m1 = gsb.tile([P, G], BF16, tag=f"m1{pt}",
                                          name=f"m1{pt}")[:, :Tg]
                            nc.vector.tensor_mul(m1, gt, l_[pt])
                            nc.vector.tensor_add(z[:, pt, :], s2[pt], m1)
                        elif part == 1:   # + tanh2 * r  (gpsimd mul)
                            m2_ = gsb.tile([P, G], BF16, tag=f"m2{pt}",
                                           name=f"m2{pt}")[:, :Tg]
                            nc.gpsimd.tensor_mul(m2_, gt, r_[pt])
                            nc.vector.tensor_add(z[:, pt, :], z[:, pt, :], m2_)
                        else:             # + tanh3 * parent (gpsimd mul)
                            m3 = gsb.tile([P, G], BF16, tag=f"m3{pt}",
                                          name=f"m3{pt}")[:, :Tg]
                            nc.gpsimd.tensor_mul(m3, gt, par[pt])
                            nc.vector.tensor_add(z[:, pt, :], z[:, pt, :], m3)
                zs = zq.tile([P, DPT, G], BF16, tag="zsq",
                             name="zsq")[:, :, :Tg]
                nc.vector.tensor_mul(zs, z, z)
                return z, zs

            # ---------- token-major tail cell (To <= 128) ----------
            def cell_tail(prev, Tin, nxt):
                Tg = Tin // 2          # output tokens this level
                hf = Tin // 2
                xk = [prev[:, 0, 0:Tg], prev[:, 1, 0:Tg],
                      prev[:, 0, hf:hf + Tg], prev[:, 1, hf:hf + Tg]]
                h_tm = tl.tile([P, 2, 512], BF16, tag="htm", name="htm")[:Tg]
                for nh in range(2):
                    hp = ps_ph.tile([P, 512], F32, tag="ph", name="hp")[:Tg]
                    for k in range(4):
                        nc.tensor.matmul(hp, lhsT=xk[k],
                                         rhs=w1s[:, k, nh * 512:(nh + 1) * 512],
                                         start=(k == 0), stop=False)
                    nc.tensor.matmul(hp, lhsT=onesr[:, :Tg],
                                     rhs=b1row[:, nh * 512:(nh + 1) * 512],
                                     start=False, stop=True)
                    nc.scalar.activation(out=h_tm[:, nh, :], in_=hp,
                                         func=AF.Gelu)
                hT = tl.tile([P, 8, P], BF16, tag="hT", name="hT")[:, :, :Tg]
                for half in range(2):
                    tp = ps_bc.tile([P, 4, P], BF16, tag="bc",
                                    name="tp")[:, :, :Tg]
                    for j in range(4):
                        nc.tensor.transpose(tp[:, j, :],
                                            h_tm[:, half, j * P:(j + 1) * P],
                                            eyes[:Tg, :Tg])
                    nc.scalar.copy(out=hT[:, half * 4:(half + 1) * 4, :],
                                   in_=tp)
                lrp = ps_bc.tile([P, 512], BF16, tag="bc", name="lrp")[:Tg]
                for i4 in range(4):
                    nc.tensor.transpose(lrp[:, i4 * P:(i4 + 1) * P], xk[i4],
                                        eyes)
                lr_tm = tl.tile([P, 512], BF16, tag="lrtm", name="lrtm")[:Tg]
                nc.scalar.copy(out=lr_tm, in_=lrp)
                gt_tm = tl.tile([P, 512], BF16, tag="gttm", name="gttm")[:Tg]
                t2d = tl.tile([P, D], BF16, tag="t2d", name="t2d")[:Tg]
                par_tm = tl.tile([P, D], BF16, tag="partm", name="partm")[:Tg]
                for nh in (1, 0):
                    cp = ps_pc.tile([P, 512], F32, tag="pc", name="cp")[:Tg]
                    for k in range(8):
                        nc.tensor.matmul(cp, lhsT=hT[:, k, :],
                                         rhs=w2s[:, k, nh * 512:(nh + 1) * 512],
                                         start=(k == 0), stop=False)
                    nc.tensor.matmul(cp, lhsT=onesr[:, :Tg],
                                     rhs=b2row[:, nh * 512:(nh + 1) * 512],
                                     start=False, stop=True)
                    if nh == 1:  # [i | parent]
                        nc.scalar.activation(out=gt_tm[:, 256:512],
                                             in_=cp[:, 0:256], func=AF.Tanh,
                                             scale=0.5)
                        nc.scalar.copy(out=par_tm, in_=cp[:, 256:512])
                    else:        # [f1 | f2]
                        nc.scalar.activation(out=gt_tm[:, 0:256],
                                             in_=cp[:, 0:256], func=AF.Tanh,
                                             scale=0.5)
                        nc.scalar.activation(out=t2d, in_=cp[:, 256:512],
                                             func=AF.Tanh, scale=0.5)
                lt = lr_tm[:, 0:256]
                rt = lr_tm[:, 256:512]
                zt = tl.tile([P, 2, D], BF16, tag="ztm", name="ztm")[:Tg]
                z_ = zt[:, 0, :]
                w_ = zt[:, 1, :]
                nc.vector.tensor_add(z_, lt, rt)
                nc.vector.tensor_add(z_, z_, par_tm)
                nc.vector.tensor_mul(w_, gt_tm[:, 0:256], lt)
                nc.vector.tensor_add(z_, z_, w_)
                nc.vector.tensor_mul(w_, t2d, rt)
                nc.vector.tensor_add(z_, z_, w_)
                nc.vector.tensor_mul(w_, gt_tm[:, 256:512], par_tm)
                nc.vector.tensor_add(z_, z_, w_)
                bst = rows.tile([P, 6], F32, tag="bst", name="bst")[:Tg]
                nc.vector.bn_stats(out=bst, in_=z_)
                mv = rows.tile([P, 2], F32, tag="mv", name="mv")[:Tg]
                nc.vector.bn_aggr(out=mv, in_=bst)
                v3 = rows.tile([P, 1], F32, tag="v3t", name="v3t")[:Tg]
                nc.vector.tensor_scalar(out=v3, in0=mv[:, 1:2], scalar1=EPS,
                                        scalar2=None, op0=ALU.add)
                sd = rows.tile([P, 1], F32, tag="sdt", name="sdt")[:Tg]
                nc.vector.tensor_scalar(
                    out=sd.bitcast(U32), in0=v3.bitcast(U32),
                    scalar1=1, scalar2=0xFFFFFFFF,
                    op0=ALU.logical_shift_right, op1=ALU.bitwise_xor)
                u_ = rows.tile([P, 1], F32, tag="u_t", name="u_t")[:Tg]
                t_ = rows.tile([P, 1], F32, tag="t_t", name="t_t")[:Tg]
                q_ = rows.tile([P, 1], F32, tag="q_t", name="q_t")[:Tg]
                rs = rows.tile([P, 1], F32, tag="rst", name="rst")[:Tg]
                nc.vector.tensor_scalar(out=sd, in0=sd, scalar1=RC,
                                        scalar2=None, op0=ALU.mult)
                nc.vector.tensor_mul(u_, sd, sd)
                nc.vector.tensor_mul(t_, v3, u_)
                nc.vector.tensor_scalar(out=q_, in0=t_, scalar1=-0.5,
                                        scalar2=1.5, op0=ALU.mult,
                                        op1=ALU.add)
                nc.vector.tensor_mul(rs, q_, sd)
                nc.vector.tensor_mul(u_, rs, rs)
                nc.vector.tensor_mul(t_, v3, u_)
                nc.vector.tensor_scalar(out=q_, in0=t_, scalar1=-0.5,
                                        scalar2=1.5, op0=ALU.mult,
                                        op1=ALU.add)
                nc.vector.tensor_mul(rs, q_, rs)
                zl = tl.tile([P, D], BF16, tag="zl", name="zl")[:Tg]
                nc.vector.tensor_scalar(out=zl, in0=z_, scalar1=mv[:, 0:1],
                                        scalar2=rs, op0=ALU.subtract,
                                        op1=ALU.mult)
                nc.vector.tensor_mul(zl, zl, gcrep[:Tg])
                nc.vector.tensor_add(zl, zl, bcrep[:Tg])
                To2 = Tg // 2
                for pt in range(DPT):
                    zpv = ps_bc.tile([P, P], BF16, tag="bc", name="zpv")[:, :Tg]
                    nc.tensor.transpose(zpv, zl[:, pt * P:(pt + 1) * P],
                                        eyes[:Tg, :Tg])
                    nc.scalar.copy(out=nxt[:, pt, 0:To2], in_=zpv[:, 0:Tg:2])
                    nc.scalar.copy(out=nxt[:, pt, To2:2 * To2],
                                   in_=zpv[:, 1:Tg:2])

            # ---------- schedule ----------
            n_sub = T0 // G
            for s in range(n_sub):
                z0, zs0 = s0_unit(s)
                if batch_full():
                    flush_batch()
                add_item(z0, zs0, 0, G, mk_writer(z0, s0out, T0, s, G, 0))

            def dump_probe(ap_list):
                for i, a in enumerate(ap_list):
                    nc.sync.dma_start(out=probe_d[:, i, :], in_=a)

            if PROBE == "s0":
                flush_batch()
                dump_probe([s0out[:, 0, 0:512], s0out[:, 1, 0:512],
                            s0out[:, 0, T0 // 2:T0 // 2 + 512],
                            s0out[:, 1, T0 // 2:T0 // 2 + 512]])

            prev, Tin = s0out, T0
            bufs = [sA, sB]
            lev = 0
            while Tin // 2 >= 256:
                To = Tin // 2
                nxt = bufs[lev % 2]
                Tg = min(G, To)
                ngroups = To // Tg
                for g in range(ngroups):
                    z, zs = cell_M(prev, Tin, g, Tg)
                    if batch_full():
                        flush_batch()
                    add_item(z, zs, 1, Tg,
                             mk_writer(z, nxt, To, g, Tg, 1))
                flush_batch()
                prev, Tin = nxt, To
                lev += 1

            while Tin > 2:
                To = Tin // 2
                nxt = bufs[lev % 2]
                cell_tail(prev, Tin, nxt)
                prev, Tin = nxt, To
                lev += 1

            outt = gsb.tile([R, D], F32, tag="outt", name="outt")
            for pt in range(DPT):
                otp = ps_st.tile([R, P], BF16, tag="st", name="otp")
                nc.tensor.transpose(otp, prev[:, pt, 0:R], eyes)
                nc.vector.tensor_copy(out=outt[:, pt * P:(pt + 1) * P],
                                      in_=otp)
            nc.sync.dma_start(out=out_d, in_=outt)

        if iters == 1:
            body()
        else:
            with tc.For_i(0, iters, 1):
                body()

        for p_ in (ps_bc, ps_st, ps_pc, ps_ph, tl, rows, ch, zq, zp, gsb,
                   hsb, xin, stp, cst):
            p_.release()

    nc.compile()
    return nc


def _prep_weights(w_word, b_word, w1, bias1, w2, bias2,
                  ln0_g, ln0_b, lnc_g, lnc_b):
    f = np.float32
    bf = ml_dtypes.bfloat16
    w1h = np.ascontiguousarray(
        w1.reshape(4, P, H).transpose(1, 0, 2), dtype=bf)
    w2h = np.ascontiguousarray(
        w2.reshape(8, P, H).transpose(1, 0, 2), dtype=bf)
    wwh = np.ascontiguousarray(
        w_word.reshape(DPT, P, D).transpose(1, 0, 2), dtype=bf)
    b1h = np.ascontiguousarray(bias1.reshape(8, P).T, dtype=f)
    b2h = np.ascontiguousarray(bias2.reshape(8, P).T, dtype=f)
    b2half = np.ascontiguousarray(0.5 * bias2.reshape(8, P).T, dtype=f)
    bwh = np.ascontiguousarray(b_word.reshape(DPT, P).T, dtype=f)
    lnrow4 = np.stack([ln0_g, -ln0_g, lnc_g, -lnc_g]).astype(np.float32)
    lnrow = np.zeros((4, 65, D), dtype=bf)
    for b in (0, 32, 64):
        lnrow[:, b, :] = lnrow4
    lnbcol = np.ascontiguousarray(
        np.stack([ln0_b, lnc_b]).reshape(2, 2, P).transpose(2, 0, 1), dtype=f)
    return dict(
        w1t=w1h, w2t=w2h, wwt=wwh, b1c=b1h, b2c=b2h, b2h=b2half, bwc=bwh,
        lnrow=lnrow, lnbcol=lnbcol,
        onescol=np.ones((P, 1), bf), eye=np.eye(P, dtype=bf),
        onesrow=np.ones((1, P), bf),
        b1row=np.ascontiguousarray(bias1.reshape(1, H), dtype=bf),
        b2row=np.ascontiguousarray(bias2.reshape(1, H), dtype=bf),
        gcrep=np.ascontiguousarray(np.broadcast_to(lnc_g, (P, D)), dtype=bf),
        bcrep=np.ascontiguousarray(np.broadcast_to(lnc_b, (P, D)), dtype=bf))


_NC_CACHE = {}


def _get_nc(S=S_FULL, iters=1):
    key = (S, iters)
    if key not in _NC_CACHE:
        _NC_CACHE[key] = _build(S, iters)
    return _NC_CACHE[key]


def kernel(input, input_mask, w_word, b_word, w1, bias1, w2, bias2,
           ln0_g, ln0_b, lnc_g, lnc_b, _iters=1):
    bf = ml_dtypes.bfloat16
    inp = np.asarray(input, dtype=np.float32)
    shared = _prep_weights(
        np.asarray(w_word), np.asarray(b_word), np.asarray(w1),
        np.asarray(bias1), np.asarray(w2), np.asarray(bias2),
        np.asarray(ln0_g), np.asarray(ln0_b), np.asarray(lnc_g),
        np.asarray(lnc_b))
    S = inp.shape[1]
    T0 = R * S
    nc = _get_nc(S, _iters)
    in_maps = []
    for c in range(N_CORES):
        m = dict(shared)
        xc = inp[c * R:(c + 1) * R].reshape(T0, D).T    # [D, T0] feature-major
        m["x"] = np.ascontiguousarray(xc.reshape(DPT, P, T0), dtype=bf)
        in_maps.append(m)
    res = run_bass_kernel_spmd(nc, in_maps, core_ids=list(range(N_CORES)))
    return np.concatenate([res.results[c]["out"] for c in range(N_CORES)],
                          axis=0)
